# revision 15
# baseline (speedup 1.0000x reference)
"""Trainium2 Bass kernel for the Correlation module.

reference:
    affinities = einsum('lnd,ond->lon', x, upfold) / sqrt(d)   # [L,O,N]
    features   = einsum('lon,ond->lnd', sigmoid(affinities)-0.5, upfold)

Math used here: sigmoid(a)-0.5 = 0.5*tanh(a/2), so with s = 1/sqrt(64):
    W^T = tanh(A^T / 16)            (A = x @ upfold^T per n)
    F   = 0.5 * (W @ upfold)        (0.5 folded into the PSUM->SBUF copy)

Datapath is bf16 (f32r matmuls run in slow fp32-HIGH mode on TRN2).
All data movement runs on the DMA engines, keeping the compute engines
free: GpSimd-issued casting DMAs convert f32->bf16 into DRAM scratch,
the DMA xbar transposes scratch into [d, l]-major SBUF tiles, and the
output takes the reverse path (cast-to-bf16 copy off PSUM is the only
engine step). The PE runs ONLY the 256 real matmuls; ACT only tanh.

Sharding: data-parallel over N across 8 cores (8 n per core), processed
as 4 pairs of n so that PE row tiling packs two K=64 matmuls (mm1) into
the full 128-row array. mm2 runs per-n with M=64 into separate PSUM
accumulators via persistent zero-padded stationaries (zero halves are
memset once; only data halves are rewritten per pair).

Self-contained: hardcodes shapes; no reads of /root/problem/*.
"""

import numpy as np

L, N, D, O = 1024, 64, 64, 1024
NCORES = 8
NLOC = N // NCORES   # 8 n per core
NPAIRS = NLOC // 2   # 4 pairs

_CACHE = {}


def _build_program():
    import concourse.mybir as mybir
    import concourse.tile as tile
    from concourse import bacc

    f32 = mybir.dt.float32
    bf16 = mybir.dt.bfloat16
    TANH = mybir.ActivationFunctionType.Tanh

    nc = bacc.Bacc(
        "TRN2", target_bir_lowering=False, debug=False, num_devices=NCORES
    )
    x_ap = nc.dram_tensor("x", [L, NLOC, D], f32, kind="ExternalInput").ap()
    u_ap = nc.dram_tensor("upfold", [O, NLOC, D], f32, kind="ExternalInput").ap()
    o_ap = nc.dram_tensor("out", [L, NLOC, D], f32, kind="ExternalOutput").ap()

    # per-pair DRAM scratch (bf16): cast-DMA targets / xbar-transpose sources
    xscr = [nc.dram_tensor(f"xscr{p}", [128, 8, 128], bf16, kind="Internal").ap()
            for p in range(NPAIRS)]
    uscr = [nc.dram_tensor(f"uscr{p}", [128, 8, 128], bf16, kind="Internal").ap()
            for p in range(NPAIRS)]
    oscr = [nc.dram_tensor(f"oscr{p}", [128, 1024], bf16, kind="Internal").ap()
            for p in range(NPAIRS)]

    with tile.TileContext(nc) as tc:
        with (
            tc.tile_pool(name="const", bufs=1) as constp,
            tc.tile_pool(name="io", bufs=2) as iop,
            tc.tile_pool(name="bfp", bufs=2) as bfp,
            tc.tile_pool(name="ost", bufs=2) as ostp,
            tc.tile_pool(name="tsp", bufs=2) as tsp,
            tc.tile_pool(name="wt", bufs=3) as wtp,
            tc.tile_pool(name="fsb", bufs=2) as fsbp,
            tc.tile_pool(name="tob", bufs=2) as tobp,
            tc.tile_pool(name="atps", bufs=3, space="PSUM") as atps,
            tc.tile_pool(name="ftps", bufs=1, space="PSUM") as ftps,
        ):
            # persistent ping-pong zero-padded mm2 stationaries:
            # uza = [0.5*U_n1 | 0], uzb = [0 | 0.5*U_n2] per o-chunk.
            # Zero halves are written once here and never touched again.
            uzs = []
            for ping in range(2):
                uza = constp.tile([128, 8, 128], bf16, name=f"uza{ping}")
                uzb = constp.tile([128, 8, 128], bf16, name=f"uzb{ping}")
                nc.gpsimd.memset(uza[:], 0.0)
                nc.gpsimd.memset(uzb[:], 0.0)
                uzs.append((uza, uzb))

            loaded = {}
            staged = {}

            def load_pair(p):
                """Half-granular f32 loads so casts can start earlier."""
                n0 = 2 * p
                halves = []
                for src_ap, tagb in ((x_ap, "xp"), (u_ap, "up")):
                    full = src_ap[:, n0 : n0 + 2, :].rearrange(
                        "(lc q) n d -> q lc (n d)", q=128
                    )
                    ha = iop.tile([128, 4, 128], f32, tag=tagb + "a")
                    nc.sync.dma_start(ha[:], full[:, 0:4, :])
                    hb = iop.tile([128, 4, 128], f32, tag=tagb + "b")
                    nc.sync.dma_start(hb[:], full[:, 4:8, :])
                    halves.extend([ha, hb])
                staged[p] = halves

            def prep_pair(p):
                """Cast f32->bf16, bounce through DRAM scratch, and
                xbar-transpose into XT/UT ([d-pair, l/o] major); refresh
                the mm2 stationaries from the bf16 u tiles."""
                xpa, xpb, upa, upb = staged.pop(p)
                XT = tsp.tile([128, 8, 128], bf16, tag="XT")
                UT = tsp.tile([128, 8, 128], bf16, tag="UT")
                xeng = nc.vector if p == 0 else nc.gpsimd
                ubs = []
                for g, (src, scr, dst, tag, eng) in enumerate((
                    (xpa, xscr[p], XT, "xba", xeng),
                    (upa, uscr[p], UT, "uba", nc.gpsimd),
                    (xpb, xscr[p], XT, "xbb", xeng),
                    (upb, uscr[p], UT, "ubb", nc.gpsimd),
                )):
                    h = slice(4 * (g // 2), 4 * (g // 2) + 4)
                    b = bfp.tile([128, 4, 128], bf16, tag=tag)
                    eng.tensor_copy(b[:], src[:])
                    nc.sync.dma_start(scr[:, h, :], b[:])
                    nc.sync.dma_start_transpose(
                        dst[:, h, :], scr[:, h, :].rearrange("q a b -> q (a b)")
                    )
                    if tag.startswith("u"):
                        ubs.append(b)
                uza, uzb = uzs[p % 2]
                for g, uh in enumerate(ubs):
                    s = slice(4 * g, 4 * g + 4)
                    nc.vector.tensor_scalar_mul(uza[:, s, 0:64], uh[:, :, 0:64], 0.5)
                    nc.vector.tensor_scalar_mul(uzb[:, s, 64:128], uh[:, :, 64:128], 0.5)
                loaded[p] = (XT, UT, uza, uzb)

            def emit_out(p, fsb):
                # bounce bf16 F^T through scratch, xbar-transpose back,
                # cast to f32 on DVE, store
                nc.sync.dma_start(oscr[p][:], fsb[:])
                tob = tobp.tile([128, 8, 128], bf16, tag="tob")
                nc.sync.dma_start_transpose(tob[:], oscr[p][:])
                ost = ostp.tile([128, 8, 128], f32, tag="ost")
                nc.vector.tensor_copy(ost[:], tob[:])
                n0 = 2 * p
                dst = o_ap[:, n0 : n0 + 2, :].rearrange(
                    "(lc q) n d -> q lc (n d)", q=128
                )
                nc.sync.dma_start(dst[:], ost[:])

            def oc_loop(p, carry):
                XT, UT, uza, uzb = loaded.pop(p)
                ft = ftps.tile([128, 1024], f32, tag="ft")

                def mm1_half(oc, ni, at):
                    rows = slice(64 * ni, 64 * (ni + 1))
                    for lh in range(2):
                        nc.tensor.matmul(
                            at[:, 512 * lh : 512 * (lh + 1)],
                            UT[rows, oc, :],
                            XT[rows, 4 * lh : 4 * lh + 4, :],
                            start=True,
                            stop=True,
                            tile_position=(64 * ni, 0),
                        )

                def mm2_half(oc, ni, w):
                    uzt = uza if ni == 0 else uzb
                    for lh in range(2):
                        nc.tensor.matmul(
                            ft[:, 512 * lh : 512 * (lh + 1)],
                            uzt[:, oc, :],
                            w[:, 512 * lh : 512 * (lh + 1)],
                            start=(oc == 0 and ni == 0),
                            stop=(oc == 7 and ni == 1),
                        )

                prev = None  # (oc, w0, w1) awaiting mm2
                pending = None
                for oc in range(8):
                    if oc == 1 and carry is not None:
                        pending = carry["fsb"]()
                    if oc == 2 and p + 1 < NPAIRS:
                        load_pair(p + 1)
                    if oc == 3 and pending is not None:
                        emit_out(*pending)
                    if oc == 4 and p + 1 < NPAIRS:
                        prep_pair(p + 1)
                    at0 = atps.tile([128, 1024], f32, tag="at")
                    at1 = atps.tile([128, 1024], f32, tag="at")
                    # PE stream: mm1(at0) | mm2(prev,ni0) | mm1(at1) | mm2(prev,ni1)
                    mm1_half(oc, 0, at0)
                    if prev is not None:
                        mm2_half(prev[0], 0, prev[1])
                    elif oc == 0 and carry is not None:
                        carry["mm2a"]()
                    mm1_half(oc, 1, at1)
                    if prev is not None:
                        mm2_half(prev[0], 1, prev[2])
                    elif oc == 0 and carry is not None:
                        carry["mm2b"]()
                    w0 = wtp.tile([128, 1024], bf16, tag="w0")
                    nc.scalar.activation(w0[:], at0[:], TANH, scale=1.0 / 16.0)
                    w1 = wtp.tile([128, 1024], bf16, tag="w1")
                    nc.scalar.activation(w1[:], at1[:], TANH, scale=1.0 / 16.0)
                    prev = (oc, w0, w1)

                def make_fsb():
                    fsb = fsbp.tile([128, 1024], bf16, name="fsb")
                    nc.vector.tensor_copy(fsb[:], ft[:])
                    return (p, fsb)

                return {
                    "mm2a": lambda: mm2_half(7, 0, prev[1]),
                    "mm2b": lambda: mm2_half(7, 1, prev[2]),
                    "fsb": make_fsb,
                }

            load_pair(0)
            prep_pair(0)
            carry = None
            for p in range(NPAIRS):
                carry = oc_loop(p, carry)
            carry["mm2a"]()
            carry["mm2b"]()
            emit_out(*carry["fsb"]())

    nc.compile()
    return nc


def _get_program():
    if "nc" not in _CACHE:
        _CACHE["nc"] = _build_program()
    return _CACHE["nc"]


def _make_in_maps(x, upfold):
    x = np.asarray(x, dtype=np.float32)
    upfold = np.asarray(upfold, dtype=np.float32)
    in_maps = []
    for c in range(NCORES):
        s = slice(NLOC * c, NLOC * (c + 1))
        in_maps.append(
            {
                "x": np.ascontiguousarray(x[:, s, :]),
                "upfold": np.ascontiguousarray(upfold[:, s, :]),
            }
        )
    return in_maps


def run_sharded(x, upfold, trace=False, **kwargs):
    """Run on all 8 cores; returns (full_output, BassKernelResults)."""
    from concourse.bass_utils import run_bass_kernel_spmd

    nc = _get_program()
    res = run_bass_kernel_spmd(
        nc, _make_in_maps(x, upfold), core_ids=list(range(NCORES)),
        trace=trace, **kwargs
    )
    out = np.concatenate([res.results[c]["out"] for c in range(NCORES)], axis=1)
    return out, res


def kernel(x, upfold):
    out, _ = run_sharded(x, upfold)
    return out


# revision 21
# speedup vs baseline: 1.1664x; 1.1664x over previous
"""Trainium2 Bass kernel for the Correlation module.

reference:
    affinities = einsum('lnd,ond->lon', x, upfold) / sqrt(d)   # [L,O,N]
    features   = einsum('lon,ond->lnd', sigmoid(affinities)-0.5, upfold)

Math used here: sigmoid(a)-0.5 = 0.5*tanh(a/2), so with s = 1/sqrt(64):
    W^T = tanh(A^T / 16)            (A = x @ upfold^T per n)
    F   = 0.5 * (W @ upfold)        (0.5 folded into the PSUM->SBUF copy)

Datapath is bf16 (f32r matmuls run in slow fp32-HIGH mode on TRN2).
All data movement runs on the DMA engines, keeping the compute engines
free: GpSimd-issued casting DMAs convert f32->bf16 into DRAM scratch,
the DMA xbar transposes scratch into [d, l]-major SBUF tiles, and the
output takes the reverse path (cast-to-bf16 copy off PSUM is the only
engine step). The PE runs ONLY the 256 real matmuls; ACT only tanh.

Sharding: data-parallel over N across 8 cores (8 n per core), processed
as 4 pairs of n so that PE row tiling packs two K=64 matmuls (mm1) into
the full 128-row array. mm2 runs per-n with M=64 into separate PSUM
accumulators via persistent zero-padded stationaries (zero halves are
memset once; only data halves are rewritten per pair).

Self-contained: hardcodes shapes; no reads of /root/problem/*.
"""

import numpy as np

L, N, D, O = 1024, 64, 64, 1024
NCORES = 8
NLOC = N // NCORES   # 8 n per core
NPAIRS = NLOC // 2   # 4 pairs

_CACHE = {}


def _build_program():
    import concourse.mybir as mybir
    import concourse.tile as tile
    from concourse import bacc

    f32 = mybir.dt.float32
    bf16 = mybir.dt.bfloat16
    TANH = mybir.ActivationFunctionType.Tanh

    nc = bacc.Bacc(
        "TRN2", target_bir_lowering=False, debug=False, num_devices=NCORES
    )
    x_ap = nc.dram_tensor("x", [L, NLOC, D], f32, kind="ExternalInput").ap()
    u_ap = nc.dram_tensor("upfold", [O, NLOC, D], f32, kind="ExternalInput").ap()
    o_ap = nc.dram_tensor("out", [L, NLOC, D], f32, kind="ExternalOutput").ap()

    # per-pair DRAM scratch (bf16): cast-DMA targets / xbar-transpose sources
    xscr = [nc.dram_tensor(f"xscr{p}", [128, 8, 128], bf16, kind="Internal").ap()
            for p in range(NPAIRS)]
    uscr = [nc.dram_tensor(f"uscr{p}", [128, 8, 128], bf16, kind="Internal").ap()
            for p in range(NPAIRS)]
    oscr = [nc.dram_tensor(f"oscr{p}", [128, 1024], bf16, kind="Internal").ap()
            for p in range(NPAIRS)]

    with tile.TileContext(nc) as tc:
        with (
            tc.tile_pool(name="const", bufs=1) as constp,
            tc.tile_pool(name="io", bufs=4) as iop,
            tc.tile_pool(name="bfp", bufs=2) as bfp,
            tc.tile_pool(name="ost", bufs=2) as ostp,
            tc.tile_pool(name="tsp", bufs=2) as tsp,
            tc.tile_pool(name="wt", bufs=3) as wtp,
            tc.tile_pool(name="fsb", bufs=2) as fsbp,
            tc.tile_pool(name="tob", bufs=2) as tobp,
            tc.tile_pool(name="atps", bufs=3, space="PSUM") as atps,
            tc.tile_pool(name="ftps", bufs=1, space="PSUM") as ftps,
        ):
            # persistent ping-pong zero-padded mm2 stationaries:
            # uza = [0.5*U_n1 | 0], uzb = [0 | 0.5*U_n2] per o-chunk.
            # Zero halves are written once here and never touched again.
            uzs = []
            for ping in range(2):
                uza = constp.tile([128, 8, 128], bf16, name=f"uza{ping}")
                uzb = constp.tile([128, 8, 128], bf16, name=f"uzb{ping}")
                nc.gpsimd.memset(uza[:], 0.0)
                nc.gpsimd.memset(uzb[:], 0.0)
                uzs.append((uza, uzb))

            loaded = {}
            staged = {}

            def load_pair(p):
                """Half-granular f32 loads so casts can start earlier."""
                n0 = 2 * p
                halves = []
                for src_ap, tagb in ((x_ap, "xp"), (u_ap, "up")):
                    full = src_ap[:, n0 : n0 + 2, :].rearrange(
                        "(lc q) n d -> q lc (n d)", q=128
                    )
                    ha = iop.tile([128, 4, 128], f32, tag=tagb + "a")
                    nc.sync.dma_start(ha[:], full[:, 0:4, :])
                    hb = iop.tile([128, 4, 128], f32, tag=tagb + "b")
                    nc.sync.dma_start(hb[:], full[:, 4:8, :])
                    halves.extend([ha, hb])
                staged[p] = halves

            def prep_pair(p):
                """Cast f32->bf16, bounce through DRAM scratch, and
                xbar-transpose into XT/UT ([d-pair, l/o] major); refresh
                the mm2 stationaries from the bf16 u tiles."""
                xpa, xpb, upa, upb = staged.pop(p)
                XT = tsp.tile([128, 8, 128], bf16, tag="XT")
                UT = tsp.tile([128, 8, 128], bf16, tag="UT")
                ubs = []
                for g, (src, scr, dst, tag, eng) in enumerate((
                    (xpa, xscr[p], XT, "xba", nc.vector),
                    (upa, uscr[p], UT, "uba", nc.gpsimd),
                    (xpb, xscr[p], XT, "xbb", nc.vector),
                    (upb, uscr[p], UT, "ubb", nc.gpsimd),
                )):
                    h = slice(4 * (g // 2), 4 * (g // 2) + 4)
                    b = bfp.tile([128, 4, 128], bf16, tag=tag)
                    eng.tensor_copy(b[:], src[:])
                    nc.sync.dma_start(scr[:, h, :], b[:])
                    nc.sync.dma_start_transpose(
                        dst[:, h, :], scr[:, h, :].rearrange("q a b -> q (a b)")
                    )
                    if tag.startswith("u"):
                        ubs.append(b)
                uza, uzb = uzs[p % 2]
                for g, uh in enumerate(ubs):
                    s = slice(4 * g, 4 * g + 4)
                    nc.vector.tensor_scalar_mul(uza[:, s, 0:64], uh[:, :, 0:64], 0.5)
                    nc.vector.tensor_scalar_mul(uzb[:, s, 64:128], uh[:, :, 64:128], 0.5)
                loaded[p] = (XT, UT, uza, uzb)

            def emit_out(p, fsb, split=False):
                # bounce bf16 F^T through scratch, xbar-transpose back,
                # cast to f32 on DVE, store; `split` pipelines the tail
                n0 = 2 * p
                dst = o_ap[:, n0 : n0 + 2, :].rearrange(
                    "(lc q) n d -> q lc (n d)", q=128
                )
                tob = tobp.tile([128, 8, 128], bf16, tag="tob")
                ost = ostp.tile([128, 8, 128], f32, tag="ost")
                for h in (slice(0, 4), slice(4, 8)) if split else (slice(0, 8),):
                    cols = slice(128 * h.start, 128 * h.stop)
                    nc.sync.dma_start(oscr[p][:, cols], fsb[:, cols])
                    nc.sync.dma_start_transpose(tob[:, h, :], oscr[p][:, cols])
                    nc.vector.tensor_copy(ost[:, h, :], tob[:, h, :])
                    nc.sync.dma_start(dst[:, h, :], ost[:, h, :])

            def oc_loop(p, carry):
                XT, UT, uza, uzb = loaded.pop(p)
                ft = ftps.tile([128, 1024], f32, tag="ft")

                def mm1_half(oc, ni, at):
                    rows = slice(64 * ni, 64 * (ni + 1))
                    for lh in range(2):
                        nc.tensor.matmul(
                            at[:, 512 * lh : 512 * (lh + 1)],
                            UT[rows, oc, :],
                            XT[rows, 4 * lh : 4 * lh + 4, :],
                            start=True,
                            stop=True,
                            tile_position=(64 * ni, 0),
                        )

                def mm2_half(oc, ni, w):
                    uzt = uza if ni == 0 else uzb
                    for lh in range(2):
                        nc.tensor.matmul(
                            ft[:, 512 * lh : 512 * (lh + 1)],
                            uzt[:, oc, :],
                            w[:, 512 * lh : 512 * (lh + 1)],
                            start=(oc == 0 and ni == 0),
                            stop=(oc == 7 and ni == 1),
                        )

                prev = None  # (oc, w0, w1) awaiting mm2
                pending = None
                for oc in range(8):
                    if oc == 1 and carry is not None:
                        pending = carry["fsb"]()
                    if oc == 1 and p + 1 < NPAIRS:
                        prep_pair(p + 1)
                    if oc == 3 and pending is not None:
                        emit_out(*pending)
                    at0 = atps.tile([128, 1024], f32, tag="at")
                    at1 = atps.tile([128, 1024], f32, tag="at")
                    # PE stream: mm1(at0) | mm2(prev,ni0) | mm1(at1) | mm2(prev,ni1)
                    mm1_half(oc, 0, at0)
                    if prev is not None:
                        mm2_half(prev[0], 0, prev[1])
                    elif oc == 0 and carry is not None:
                        carry["mm2a"]()
                    mm1_half(oc, 1, at1)
                    if prev is not None:
                        mm2_half(prev[0], 1, prev[2])
                    elif oc == 0 and carry is not None:
                        carry["mm2b"]()
                    w0 = wtp.tile([128, 1024], bf16, tag="w0")
                    nc.scalar.activation(w0[:], at0[:], TANH, scale=1.0 / 16.0)
                    w1 = wtp.tile([128, 1024], bf16, tag="w1")
                    nc.scalar.activation(w1[:], at1[:], TANH, scale=1.0 / 16.0)
                    prev = (oc, w0, w1)

                def make_fsb():
                    fsb = fsbp.tile([128, 1024], bf16, name="fsb")
                    nc.vector.tensor_copy(fsb[:], ft[:])
                    return (p, fsb)

                return {
                    "mm2a": lambda: mm2_half(7, 0, prev[1]),
                    "mm2b": lambda: mm2_half(7, 1, prev[2]),
                    "fsb": make_fsb,
                }

            for p in range(NPAIRS):
                load_pair(p)
            prep_pair(0)
            carry = None
            for p in range(NPAIRS):
                carry = oc_loop(p, carry)
            carry["mm2a"]()
            carry["mm2b"]()
            emit_out(*carry["fsb"](), split=True)

    nc.compile()
    return nc


def _get_program():
    if "nc" not in _CACHE:
        _CACHE["nc"] = _build_program()
    return _CACHE["nc"]


def _make_in_maps(x, upfold):
    x = np.asarray(x, dtype=np.float32)
    upfold = np.asarray(upfold, dtype=np.float32)
    in_maps = []
    for c in range(NCORES):
        s = slice(NLOC * c, NLOC * (c + 1))
        in_maps.append(
            {
                "x": np.ascontiguousarray(x[:, s, :]),
                "upfold": np.ascontiguousarray(upfold[:, s, :]),
            }
        )
    return in_maps


def run_sharded(x, upfold, trace=False, **kwargs):
    """Run on all 8 cores; returns (full_output, BassKernelResults)."""
    from concourse.bass_utils import run_bass_kernel_spmd

    nc = _get_program()
    res = run_bass_kernel_spmd(
        nc, _make_in_maps(x, upfold), core_ids=list(range(NCORES)),
        trace=trace, **kwargs
    )
    out = np.concatenate([res.results[c]["out"] for c in range(NCORES)], axis=1)
    return out, res


def kernel(x, upfold):
    out, _ = run_sharded(x, upfold)
    return out
